# revision 12
# baseline (speedup 1.0000x reference)
"""Bass/Tile kernel for nn_CorrOptL2SDGN: 5 steepest-descent iterations of a
ridge-regularized correlation-filter optimizer, 32 sequences data-parallel
over 8 cores (4 seqs/core).

v2 design ("batched fp16 C-major"):
  - All state (f, g, fgM, M, X) in fp16, uniformly scaled by 2^-5 so that
    every fp16 intermediate (g^2, g*fgM, fgM) stays in range; the scale
    cancels exactly in alpha = step*num/den.
  - fp16 matmuls are ~2-3x cheaper on the PE than f32r, and fp16 doubles
    DVE elementwise throughput (2x_1p mode).
  - The 4 sequences' num/den row reductions land in a single [4,484] PSUM
    tile (one-hot [128,4] stationary operands, partition s <- seq s), so the
    reciprocal/alpha row math runs once per iteration, not once per seq.
  - Elementwise ops are batched over 2-seq groups ([128, 968] tiles) to
    amortize fixed per-instruction overheads.
Math (per sequence), Gram reformulation (all tensors C-major [C, F]):
  M = X X^T + reg I; g_0 = M f + X(-T)
  per iter: fgM = M g; num_f = sum_c g^2; den_f = sum_c g*fgM
            a = step*num/den; f -= a (x)col g; g -= a (x)col fgM
"""

import sys

sys.path.insert(0, "/opt/trn_rl_repo")

from contextlib import ExitStack

import numpy as np

S_TOTAL, C, F = 32, 256, 484
NCORES = 8
SPC = S_TOTAL // NCORES
NUM_ITER = 5
HCH = [0, 121, 242, 363, 484]
CCH = [0, 128, 256]
NGRP = 2          # sequence groups per core
GW = 2 * F        # group tile width (2 seqs side by side)
SCALE = 1.0 / 32.0  # uniform fp16 state scale (2^-5)


def build(spc=SPC, num_iter=NUM_ITER):
    import concourse.bacc as bacc
    import concourse.mybir as mybir
    import concourse.tile as tile

    F32 = mybir.dt.float32
    F16 = mybir.dt.float16
    AF = mybir.ActivationFunctionType
    ALU = mybir.AluOpType

    nc = bacc.Bacc("TRN2", target_bir_lowering=False, debug=False)
    featT_d = nc.dram_tensor("featT16", [spc, F, C], F16, kind="ExternalInput")
    ftT_d = nc.dram_tensor("ftT16", [spc, C, F], F16, kind="ExternalInput")
    negT_d = nc.dram_tensor("negT16", [F, F], F16, kind="ExternalInput")
    lsl_d = nc.dram_tensor("lsl", [1], F32, kind="ExternalInput")
    reg_d = nc.dram_tensor("freg", [1], F32, kind="ExternalInput")
    sel_d = nc.dram_tensor("sel16", [spc, spc, 128], F16, kind="ExternalInput")
    out_d = nc.dram_tensor("outT", [spc, C, F], F32, kind="ExternalOutput")

    def grp(s):
        return s // 2

    def half(s):
        h = (s % 2) * F
        return slice(h, h + F)

    with ExitStack() as ctx:
        tc = ctx.enter_context(tile.TileContext(nc))
        const = ctx.enter_context(tc.tile_pool(name="const", bufs=1))
        state = ctx.enter_context(tc.tile_pool(name="state", bufs=1))
        work = ctx.enter_context(tc.tile_pool(name="work", bufs=2))
        gwork = ctx.enter_context(tc.tile_pool(name="gwork", bufs=2 * NGRP))
        psmm = ctx.enter_context(tc.tile_pool(name="psmm", bufs=3, space="PSUM"))
        psgram = ctx.enter_context(tc.tile_pool(name="psgram", bufs=1, space="PSUM"))
        psrow = ctx.enter_context(tc.tile_pool(name="psrow", bufs=2, space="PSUM"))
        psab = ctx.enter_context(tc.tile_pool(name="psab", bufs=2, space="PSUM"))

        # ---- global constants ----
        # SEL[s]: one-hot row s -> broadcast matmul out[p,f] = al16[s,f]
        SEL = []
        for s in range(spc):
            t = const.tile([spc, 128], F16, tag=f"SEL{s}")
            nc.sync.dma_start(t[:], sel_d.ap()[s])
            SEL.append(t)
        # one-hot stationary tiles: E[s] has ones in column s only, so the
        # reduction matmul for seq s lands in partition s of a [4,484] row tile
        E = []
        for s in range(spc):
            t = const.tile([128, spc], F16, tag=f"E{s}")
            nc.vector.memset(t[:], 0.0)
            nc.vector.memset(t[:, s:s + 1], 1.0)
            E.append(t)

        # step4 = exp(log_step_length), replicated on 4 partitions
        step4 = const.tile([spc, 1], F32, tag="step4")
        nc.sync.dma_start(step4[:], lsl_d.ap().to_broadcast((spc, 1)))
        nc.scalar.activation(step4[:], step4[:], AF.Exp, scale=1.0)

        reg_sb = const.tile([128, 1], F32, tag="reg_sb")
        nc.sync.dma_start(reg_sb[:], reg_d.ap().to_broadcast((128, 1)))
        nc.scalar.square(reg_sb[:], reg_sb[:])
        nc.vector.tensor_scalar_max(reg_sb[:], reg_sb[:], 1e-10)

        ones_t = const.tile([128, C], F32, tag="ones_t")
        nc.vector.memset(ones_t[:], 1.0)
        regI = []
        for c0 in range(2):
            t = const.tile([128, C], F32, tag=f"regI{c0}")
            nc.gpsimd.affine_select(
                t[:], ones_t[:], pattern=[[1, C]], base=-(c0 * 128),
                channel_multiplier=-1, compare_op=ALU.is_equal, fill=0.0)
            nc.vector.tensor_scalar_mul(t[:], t[:], reg_sb[:])
            regI.append(t)

        # ---- input DMAs ----
        AT = {}
        for h in range(4):  # seq 0 features first so its Gram starts ASAP
            t = state.tile([121, C], F16, tag=f"AT0_{h}")
            nc.sync.dma_start(t[:], featT_d.ap()[0, HCH[h]:HCH[h + 1], :])
            AT[0, h] = t
        negTr = []
        for h in range(4):
            t = const.tile([121, F], F16, tag=f"negTr{h}")
            nc.sync.dma_start(t[:], negT_d.ap()[HCH[h]:HCH[h + 1], :])
            negTr.append(t)
        for s in range(1, spc):
            for h in range(4):
                t = state.tile([121, C], F16, tag=f"AT{s}_{h}")
                nc.sync.dma_start(t[:], featT_d.ap()[s, HCH[h]:HCH[h + 1], :])
                AT[s, h] = t
        # f state, fp16, pre-scaled by 2^-5 on host, 2-seq group tiles
        Fg = {}
        for g in range(NGRP):
            for c0 in range(2):
                t = state.tile([128, GW], F16, tag=f"F{g}_{c0}")
                for sh in range(2):
                    s = 2 * g + sh
                    nc.sync.dma_start(t[:, sh * F:(sh + 1) * F],
                                      ftT_d.ap()[s, CCH[c0]:CCH[c0 + 1], :])
                Fg[g, c0] = t

        # ---- per-sequence setup: Gram matrix and g0 ----
        M16 = {}
        Gg = {}
        for g in range(NGRP):
            for c0 in range(2):
                t = state.tile([128, GW], F16, tag=f"G{g}_{c0}")
                Gg[g, c0] = t
        for s in range(spc):
            for c0 in range(2):
                pm = psgram.tile([128, C], F32, tag="gram")
                for h in range(4):
                    nc.tensor.matmul(
                        pm[:], AT[s, h][:, CCH[c0]:CCH[c0 + 1]], AT[s, h][:],
                        start=(h == 0), stop=(h == 3))
                t = state.tile([128, C], F16, tag=f"M{s}_{c0}")
                nc.vector.tensor_add(t[:], pm[:], regI[c0][:])
                M16[s, c0] = t
            # g0 = M f + X(-T)   (inherits f's 2^-5 scale from Fg; negTr is
            # scaled 2^-5 on host so both terms match)
            for c0 in range(2):
                pg = psmm.tile([128, F], F32, tag="mm")
                nc.tensor.matmul(pg[:], M16[s, 0][:, CCH[c0]:CCH[c0 + 1]],
                                 Fg[grp(s), 0][:, half(s)], start=True, stop=False)
                nc.tensor.matmul(pg[:], M16[s, 1][:, CCH[c0]:CCH[c0 + 1]],
                                 Fg[grp(s), 1][:, half(s)], start=False, stop=False)
                for h in range(4):
                    nc.tensor.matmul(pg[:], AT[s, h][:, CCH[c0]:CCH[c0 + 1]],
                                     negTr[h][:], start=False, stop=(h == 3))
                nc.scalar.copy(Gg[grp(s), c0][:, half(s)], pg[:])

        # ---- iterations ----
        for i in range(num_iter):
            last = i == num_iter - 1
            # fgM = M g (per seq, per c-chunk) -> PSUM, copy to fp16 SBUF
            fgMg = {}
            for g in range(NGRP):
                for c0 in range(2):
                    t = gwork.tile([128, GW], F16, tag="fgM")
                    fgMg[g, c0] = t
            for s in range(spc):
                for c0 in range(2):
                    p = psmm.tile([128, F], F32, tag="mm")
                    nc.tensor.matmul(p[:], M16[s, 0][:, CCH[c0]:CCH[c0 + 1]],
                                     Gg[grp(s), 0][:, half(s)], start=True, stop=False)
                    nc.tensor.matmul(p[:], M16[s, 1][:, CCH[c0]:CCH[c0 + 1]],
                                     Gg[grp(s), 1][:, half(s)], start=False, stop=True)
                    # split copies across ACT and DVE (GPSIMD can't read PSUM)
                    if s < 2:
                        nc.scalar.copy(fgMg[grp(s), c0][:, half(s)], p[:])
                    else:
                        nc.vector.tensor_copy(fgMg[grp(s), c0][:, half(s)], p[:])
            # num = step * colsum(g^2): square on DVE, reduce on PE
            pnum = psrow.tile([spc, F], F32, tag="row")
            sqg = {}
            for g in range(NGRP):
                for c0 in range(2):
                    t = gwork.tile([128, GW], F16, tag="sq")
                    nc.vector.tensor_mul(t[:], Gg[g, c0][:], Gg[g, c0][:])
                    sqg[g, c0] = t
            k = 0
            for s in range(spc):
                for c0 in range(2):
                    nc.tensor.matmul(pnum[:], E[s][:], sqg[grp(s), c0][:, half(s)],
                                     start=(k == 0), stop=(k == 2 * spc - 1))
                    k += 1
            # den = colsum(g * fgM)
            pden = psrow.tile([spc, F], F32, tag="row")
            tdg = {}
            for g in range(NGRP):
                for c0 in range(2):
                    t = gwork.tile([128, GW], F16, tag="td")
                    nc.vector.tensor_mul(t[:], Gg[g, c0][:], fgMg[g, c0][:])
                    tdg[g, c0] = t
            k = 0
            for s in range(spc):
                for c0 in range(2):
                    nc.tensor.matmul(pden[:], E[s][:], tdg[grp(s), c0][:, half(s)],
                                     start=(k == 0), stop=(k == 2 * spc - 1))
                    k += 1
            # a = step*num/den for all 4 seqs at once ([4,484] rows)
            rec4 = work.tile([spc, F], F32, tag="rec4")
            nc.vector.reciprocal_approx_fast(rec4[:], pden[:])
            al16 = work.tile([spc, F], F16, tag="al16")
            nc.vector.scalar_tensor_tensor(al16[:], pnum[:], step4[:], rec4[:],
                                           ALU.mult, ALU.mult)
            # broadcast a to 128 partitions (per seq), then to fp16 group tiles
            ab16 = {}
            for g in range(NGRP):
                t = gwork.tile([128, GW], F16, tag="ab16")
                ab16[g] = t
            for s in range(spc):
                pab = psab.tile([128, F], F32, tag="ab")
                nc.tensor.matmul(pab[:], SEL[s][:], al16[:],
                                 start=True, stop=True)
                nc.scalar.copy(ab16[grp(s)][:, half(s)], pab[:])
            # f update (and g update except on the last iteration)
            for g in range(NGRP):
                for c0 in range(2):
                    updf = gwork.tile([128, GW], F16, tag="updf")
                    nc.vector.tensor_mul(updf[:], ab16[g][:], Gg[g, c0][:])
                    if last:
                        fnew = gwork.tile([128, GW], F16, tag="fnew")
                        nc.vector.tensor_sub(fnew[:], Fg[g, c0][:], updf[:])
                        outf = gwork.tile([128, GW], F32, tag="outf")
                        # undo the 2^-5 state scale on the way out
                        nc.scalar.activation(outf[:], fnew[:], AF.Copy,
                                             scale=1.0 / SCALE)
                        for sh in range(2):
                            s = 2 * g + sh
                            nc.sync.dma_start(
                                out_d.ap()[s, CCH[c0]:CCH[c0 + 1], :],
                                outf[:, sh * F:(sh + 1) * F])
                    else:
                        nc.vector.tensor_sub(Fg[g, c0][:], Fg[g, c0][:], updf[:])
                        updg = gwork.tile([128, GW], F16, tag="updg")
                        nc.gpsimd.tensor_mul(updg[:], ab16[g][:], fgMg[g, c0][:])
                        nc.vector.tensor_sub(Gg[g, c0][:], Gg[g, c0][:], updg[:])

    nc.compile()
    return nc


def make_neg_target():
    k = np.arange(22, dtype=np.float64)
    d = (k[:, None] - k[None, :]) ** 2
    g = np.exp(-0.5 * (d[:, None, :, None] + d[None, :, None, :]))
    return (-g.reshape(F, F)).astype(np.float32)


def make_in_maps(filter, feat, log_step_length, filter_reg, ncores=NCORES, spc=SPC):
    negT16 = (make_neg_target() * SCALE).astype(np.float16)
    lsl = np.ascontiguousarray(log_step_length, np.float32)
    freg = np.ascontiguousarray(filter_reg, np.float32)
    sel16 = np.zeros((SPC, SPC, 128), np.float16)
    for s in range(SPC):
        sel16[s, s, :] = 1.0
    f = np.asarray(filter, np.float32)[:, :, :, 0, 0]
    x = np.asarray(feat, np.float32)[0].reshape(-1, C, F)
    in_maps = []
    for c in range(ncores):
        sl = slice(c * spc, (c + 1) * spc)
        in_maps.append({
            "featT16": np.ascontiguousarray(
                x[sl].transpose(0, 2, 1)).astype(np.float16),
            "ftT16": (np.ascontiguousarray(
                f[sl].transpose(0, 2, 1)) * SCALE).astype(np.float16),
            "negT16": negT16,
            "lsl": lsl,
            "freg": freg,
            "sel16": sel16,
        })
    return in_maps


def assemble_output(results, ncores=NCORES, spc=SPC):
    out = np.empty((S_TOTAL, F, C), np.float32)
    for c in range(ncores):
        out[c * spc:(c + 1) * spc] = results[c]["outT"].transpose(0, 2, 1)
    return out[:, :, :, None, None]


_nc_cache = None


from contextlib import contextmanager


@contextmanager
def _neuron_devices_visible():
    """run_bass_via_pjrt uses the default-platform jax.devices(); if a caller
    pinned jax to cpu, point jax.devices at the axon/neuron plugin for the
    duration of the call."""
    import os

    if "jax" not in sys.modules and os.environ.get("JAX_PLATFORMS") in ("cpu",):
        del os.environ["JAX_PLATFORMS"]
    import jax

    devs = jax.devices()
    if len(devs) >= NCORES and devs[0].platform != "cpu":
        yield
        return
    plat = None
    for cand in ("axon", "neuron"):
        try:
            if len(jax.devices(cand)) >= NCORES:
                plat = cand
                break
        except Exception:
            continue
    if plat is None:
        yield
        return
    real = jax.devices

    def patched(backend=None):
        return real(plat if backend is None else backend)

    jax.devices = patched
    try:
        yield
    finally:
        jax.devices = real


def kernel(filter, feat, test_anno, log_step_length, filter_reg):
    global _nc_cache
    if _nc_cache is None:
        _nc_cache = build()
    from concourse.bass_utils import run_bass_kernel_spmd

    in_maps = make_in_maps(filter, feat, log_step_length, filter_reg)
    with _neuron_devices_visible():
        res = run_bass_kernel_spmd(_nc_cache, in_maps, core_ids=list(range(NCORES)))
    return assemble_output(res.results)
